# revision 1
# baseline (speedup 1.0000x reference)
"""CRF layer (forward-algorithm NLL) on 8 Trainium2 NeuronCores.

Data-parallel over the batch: 8 cores x 32 sequences. logZ in probability
space via block decomposition: the 1024-step recurrence
    p' = diag(e_t) @ T~ @ p,     T~ = exp(trans - LNS)
contracts projectively per step, so even 4-step blocks are numerically
rank-1 (M_b ~= v_b w_b^T) and the chain stitches with per-block scalars.

Device work per core: ONLY the forward probes u_b = M_b @ 1 for 256
blocks of LB=4 steps. Step 0 collapses to s_1 = rho .* e_0 (DVE
tensor_scalar with per-partition rho = T~ @ 1, no matmul) and the last
step of each block is applied inside the host stitch (one f64 einsum),
so the device runs only steps 1..2 of every 4 — half the timestep rows
on the matmul path. Work is packed as 4 superchains of [128, 1024],
each processing two 32-block sets (g = 0, 1) sequentially: per set, one
tensor_scalar + 2 rounds of (2 matmuls N=512 into adjacent PSUM banks +
one fused multiply).

Stitching (host, f64) via depth-1-truncated backward probes, which
collapse entirely to host math (c~_b = T~^T e_{b,0}):
    u_b   = e_{b,3} .* (T~ @ u~_b)          (u~ = device state, step 2)
    num_b = e_{b,0} . (T~ u_{b-1}),  den_b = e_{b,0} . rho
    logZ  = log(beta.u_255) + log(c~_0[START]/den_0)
          + sum_{b>=1} log(num_b/den_b) + (L + 1) * LNS
(truncation exact to 4.8e-4 in f64; bf16/fp8 device noise ~0.2-0.4 abs
on outputs ~5400 vs the 2e-2-relative gate.)

Engine schedule: the matmul output must leave PSUM each step; the t=1
rounds give one superchain path A (DVE tensor_tensor reads PSUM f32
directly at 1x, so that emission slice ships as fp8), the rest use path
B (Scalar copy PSUM->SBUF bf16 ~1.1us + DVE 2x multiply ~0.6us),
balancing DVE ~15us ~= Scalar ~15us totals. Emissions are t-major so
DMA chunks arrive in consumption order, early chunks split across both
HWDGE rings; u-slabs DMA out per block-set as superchains finish.
"""

import numpy as np
import ml_dtypes

B, L, NTAG = 256, 1024, 128
NCORES = 8
SEQ = B // NCORES          # 32 sequences per core
LB = 4                     # timesteps per block
NBLK = L // LB             # 256 blocks
NSUP = 4                   # superchains
NSET = 2                   # block-sets per superchain
JB = 32                    # blocks per (superchain, set)
W2 = JB * SEQ              # 1024 columns per superchain op
TROW = NSUP * W2           # 4096 columns per (set, timestep) row
START, END = 126, 127
LNS = float(np.log(128.0) + 0.5)

# path-A (PSUM-direct) assignment: (round r, superchain) pairs; r = g*2+(t-1)
_A_STEP = {0: 0, 2: 2}     # t=1 rounds only; t=2 rounds are all path B

_PROG = None


def _build_program():
    from contextlib import ExitStack

    import concourse.bacc as bacc
    import concourse.tile as tile
    import concourse.mybir as mybir
    from concourse.alu_op_type import AluOpType

    F32 = mybir.dt.float32
    BF16 = mybir.dt.bfloat16
    FP8 = mybir.dt.float8e4
    MULT = AluOpType.mult

    nc = bacc.Bacc("TRN2", target_bir_lowering=False, debug=False)

    XT0 = nc.dram_tensor("XT0", (NTAG, NSET * TROW), BF16,
                         kind="ExternalInput")
    XT16 = nc.dram_tensor("XT16", (NTAG, 14 * W2), BF16,
                          kind="ExternalInput")
    XT8 = nc.dram_tensor("XT8", (NTAG, 2 * W2), FP8, kind="ExternalInput")
    EF = nc.dram_tensor("EF", (NTAG, NTAG), BF16, kind="ExternalInput")
    RHO = nc.dram_tensor("RHO", (NTAG, 1), F32, kind="ExternalInput")
    UOUT = nc.dram_tensor("UOUT", (NTAG, NBLK * SEQ), BF16,
                          kind="ExternalOutput")

    # B-slice counts per round r=0..3 and XT16 base offsets
    B_CNT = [3, 4, 3, 4]
    B_OFF = [0, 3, 7, 10]

    with tile.TileContext(nc) as tc, ExitStack() as ctx:
        const = ctx.enter_context(tc.tile_pool(name="const", bufs=1))
        qpool = ctx.enter_context(tc.tile_pool(name="qp", bufs=1, space="PSUM"))
        spool = ctx.enter_context(tc.tile_pool(name="sp", bufs=3))

        ef = const.tile([NTAG, NTAG], BF16, tag="ef")
        rho = const.tile([NTAG, 1], F32, tag="rho")
        nc.scalar.dma_start(ef[:], EF[:])
        nc.scalar.dma_start(rho[:], RHO[:])

        ubuf = const.tile([NTAG, NBLK * SEQ], BF16, tag="ubuf")

        e0q = [[const.tile([NTAG, W2], BF16, tag=f"e0q{g}_{h}",
                           name=f"e0q{g}_{h}") for h in range(4)]
               for g in range(NSET)]
        e16 = [const.tile([NTAG, B_CNT[r] * W2], BF16, tag=f"e16_{r}",
                          name=f"e16_{r}") for r in range(4)]
        e8 = {r: const.tile([NTAG, W2], FP8, tag=f"e8_{r}", name=f"e8_{r}")
              for r in _A_STEP}

        # DMA issue order == consumption order; earliest chunks split
        # across both HWDGE rings (sync + scalar)
        for h in range(4):
            eng = nc.sync if h < 2 else nc.scalar
            eng.dma_start(e0q[0][h][:], XT0[:, h * W2:(h + 1) * W2])
        nc.scalar.dma_start(e8[0][:], XT8[:, 0:W2])
        half = 3 * W2 // 2
        for h in range(2):
            eng = nc.scalar if h == 0 else nc.sync
            eng.dma_start(e16[0][:, h * half:(h + 1) * half],
                          XT16[:, h * half:(h + 1) * half])
        nc.sync.dma_start(e16[1][:], XT16[:, 3 * W2:7 * W2])
        for h in range(4):
            nc.sync.dma_start(e0q[1][h][:],
                              XT0[:, (4 + h) * W2:(5 + h) * W2])
        nc.sync.dma_start(e8[2][:], XT8[:, W2:2 * W2])
        nc.sync.dma_start(e16[2][:], XT16[:, 7 * W2:10 * W2])
        nc.sync.dma_start(e16[3][:], XT16[:, 10 * W2:14 * W2])

        qt = [qpool.tile([NTAG, W2], F32, tag=f"q{m}", name=f"q{m}")
              for m in range(NSUP)]

        W = W2 // 2
        for g in range(NSET):
            # step 0: s1 = rho .* e_0 on the DVE
            state = []
            for m in range(NSUP):
                st = spool.tile([NTAG, W2], BF16, tag=f"st{m}",
                                name=f"ts{g}_{m}")
                nc.vector.tensor_scalar_mul(st[:], e0q[g][m][:], rho[:, 0:1])
                state.append(st[:])

            for t in (1, 2):
                r = g * 2 + (t - 1)
                ma = _A_STEP.get(r, -1)
                order = ([ma] if ma >= 0 else []) + \
                    [x for x in range(NSUP) if x != ma]
                for m in order:
                    nc.tensor.matmul(qt[m][:, 0:W], ef[:],
                                     state[m][:, 0:W],
                                     start=True, stop=True)
                    nc.tensor.matmul(qt[m][:, W:W2], ef[:],
                                     state[m][:, W:W2],
                                     start=True, stop=True)
                    if m == ma:
                        eslice = e8[r][:]
                    else:
                        pos = m - (1 if ma >= 0 and m > ma else 0)
                        eslice = e16[r][:, pos * W2:(pos + 1) * W2]
                    if t == 2:
                        nxt = ubuf[:, (g * NSUP + m) * W2:
                                   (g * NSUP + m + 1) * W2]
                    else:
                        st = spool.tile([NTAG, W2], BF16, tag=f"st{m}",
                                        name=f"st{g}_{t}_{m}")
                        nxt = st[:]
                    if m == ma:
                        nc.vector.tensor_tensor(nxt, qt[m][:], eslice, MULT)
                    else:
                        sc = spool.tile([NTAG, W2], BF16, tag=f"sc{m}",
                                        name=f"sc{g}_{t}_{m}")
                        nc.scalar.copy(sc[:], qt[m][:])
                        nc.vector.tensor_tensor(nxt, sc[:], eslice, MULT)
                    state[m] = nxt
            for m in (0, 2):
                lo = (g * NSUP + m) * W2
                nc.sync.dma_start(UOUT[:, lo:lo + 2 * W2],
                                  ubuf[:, lo:lo + 2 * W2])

    nc.compile()
    return nc


def _get_program():
    global _PROG
    if _PROG is None:
        _PROG = _build_program()
    return _PROG


def _gold_score(X, y, trans):
    """Gold path score per sequence, float64 on host."""
    Xd = X.astype(np.float64)
    td = trans.astype(np.float64)
    yi = y.astype(np.int64)
    prev = np.concatenate(
        [np.full((B, 1), START, dtype=np.int64), yi[:, :-1]], axis=1
    )
    emit = np.take_along_axis(Xd, yi[:, :, None], axis=2)[:, :, 0]
    tr = td[yi, prev]
    return emit.sum(1) + tr.sum(1) + td[END, yi[:, -1]]


def _prep_in_maps(X, trans):
    bf16 = ml_dtypes.bfloat16
    fp8 = ml_dtypes.float8_e4m3fn
    Tm = np.exp(trans.astype(np.float64) - LNS)       # [i, j]
    efm = np.ascontiguousarray(Tm.T).astype(bf16)     # fwd lhsT
    rho = Tm.sum(axis=1).astype(np.float32)[:, None]  # T~ @ 1, [128, 1]

    E = np.exp(X.astype(np.float32))                  # [B, L, NTAG] f32
    in_maps = []
    for c in range(NCORES):
        Ec = E[c * SEQ:(c + 1) * SEQ]                 # [32, 1024, 128]
        # t_global = ((g*4 + m)*32 + j)*4 + t -> [tag, g, m, j, t, seq]
        x6 = Ec.transpose(2, 1, 0).reshape(NTAG, NSET, NSUP, JB, LB, SEQ)

        def row(g, t):
            return x6[:, g, :, :, t, :].reshape(NTAG, TROW)

        xt0 = np.ascontiguousarray(
            np.concatenate([row(0, 0), row(1, 0)], axis=1)).astype(bf16)
        # round r -> (g, t): 0:(0,1) 1:(0,2) 2:(1,1) 3:(1,2)
        r01 = row(0, 1)
        r02 = row(0, 2)
        r11 = row(1, 1)
        r12 = row(1, 2)
        xt8 = np.ascontiguousarray(np.concatenate(
            [r01[:, 0:W2],                 # r=0, ma=0
             r11[:, 2 * W2:3 * W2]],       # r=2, ma=2
            axis=1)).astype(fp8)
        xt16 = np.ascontiguousarray(np.concatenate(
            [r01[:, W2:4 * W2],                         # r=0 B: m=1,2,3
             r02,                                       # r=1 B: all
             r11[:, 0:2 * W2], r11[:, 3 * W2:4 * W2],   # r=2 B: m=0,1,3
             r12],                                      # r=3 B: all
            axis=1)).astype(bf16)
        in_maps.append({"XT0": xt0, "XT16": xt16, "XT8": xt8,
                        "EF": efm, "RHO": rho})
    return in_maps


def kernel(X, y, trans):
    from concourse import bass_utils

    nc = _get_program()
    in_maps = _prep_in_maps(X, trans)
    res = bass_utils.run_bass_kernel_spmd(
        nc, in_maps, core_ids=list(range(NCORES))
    )

    Tm = np.exp(trans.astype(np.float64) - LNS)            # [i, j]
    rho = Tm.sum(axis=1)                                   # [128]
    beta = np.exp(trans[END, :].astype(np.float64) - LNS)  # [128]
    tcol = Tm[:, START]                                    # T~[:, START]

    logZ = np.empty(B, dtype=np.float64)
    for c in range(NCORES):
        U = res.results[c]["UOUT"].astype(np.float64).reshape(
            NTAG, NBLK, SEQ)      # pos b = device state of block b (step 2)
        Xc = X[c * SEQ:(c + 1) * SEQ].astype(np.float64)   # [32, 1024, 128]
        e0 = np.exp(Xc[:, ::LB, :]).transpose(2, 1, 0)     # [tag, blk, seq]
        e3 = np.exp(Xc[:, LB - 1::LB, :]).transpose(2, 1, 0)

        # apply the absorbed last step of each block: u_b = e3 .* (T~ u~)
        Ufull = e3 * np.einsum("it,tbs->ibs", Tm, U)
        den = np.einsum("tbs,t->bs", e0, rho)              # [NBLK, SEQ]
        TU = np.einsum("it,tbs->ibs", Tm, Ufull[:, :NBLK - 1, :])
        num = np.empty_like(den)
        num[1:] = np.einsum("tbs,tbs->bs", e0[:, 1:, :], TU)
        num[0] = np.einsum("ts,t->s", e0[:, 0, :], tcol)   # c~_0 . p0
        tail = beta @ Ufull[:, NBLK - 1, :]                # [SEQ]
        lz = (np.log(tail)
              + np.log(num / den).sum(axis=0)
              + (L + 1) * LNS)
        logZ[c * SEQ:(c + 1) * SEQ] = lz

    gold = _gold_score(X, y, trans)
    return (logZ - gold).astype(np.float32)



# revision 2
# speedup vs baseline: 1.1961x; 1.1961x over previous
"""CRF layer (forward-algorithm NLL) on 8 Trainium2 NeuronCores.

Data-parallel over the batch: 8 cores x 32 sequences. logZ in probability
space via block decomposition: the 1024-step recurrence
    p' = diag(e_t) @ T~ @ p,     T~ = exp(trans - LNS)
contracts projectively per step, so 4-step blocks are numerically rank-1
(M_b ~= v_b w_b^T) and the chain stitches with per-block scalars.

Device work per core: the two interior T~-applies of each of 256 blocks,
on 8192 block-columns packed as 8 chains of [128, 1024]:
    q2 = T~ @ ((e1/2) .* (T~ @ s1)),   s1 = rho .* e0  (host-precomputed)
Per chain: 2 matmuls N=512 into PSUM, one DVE multiply (PSUM f32 x fp8
emission -> fp8), 2 more matmuls, one Scalar copy PSUM->SBUF fp8, DMA out.
All device I/O is fp8e4m3 (values scaled into [0, 240]); the stationary
T~^T is bf16. Inputs ship as one interleaved DRAM tensor in consumption
order with 4 tiered dma_starts (sizes 2,2,4,8 x [128,1024]) so the first
chain starts early while later chunks amortize issue cost; outputs leave
in 4 chunks of 2 chains.

Stitching (host, f64): block step 0 is folded into s1, steps 2 and 3 into
the stitch einsum (u_b = e3 .* (T~ @ (e2 .* 2*q2))), and block boundaries
use depth-1-truncated backward probes exactly as before:
    num_b = e_{b,0} . (T~ u_{b-1}),  den_b = e_{b,0} . rho
    logZ  = log(beta.u_255) + log(c~_0[START]/den_0)
          + sum_{b>=1} log(num_b/den_b) + (L + 1) * LNS
(truncation ~5e-4; fp8 device noise ~2 abs on outputs ~5400 vs the
2e-2-relative gate.)
"""

import numpy as np
import ml_dtypes

B, L, NTAG = 256, 1024, 128
NCORES = 8
SEQ = B // NCORES          # 32 sequences per core
LB = 4                     # timesteps per block
NBLK = L // LB             # 256 blocks
NCH = 8                    # chains of [128, 1024] per core
W = 1024                   # columns per chain
HW = 512                   # matmul free dim (one PSUM bank)
START, END = 126, 127
LNS = float(np.log(128.0) + 0.5)

_PROG = None


def _build_program():
    from contextlib import ExitStack

    import concourse.bacc as bacc
    import concourse.tile as tile
    import concourse.mybir as mybir
    from concourse.alu_op_type import AluOpType

    F32 = mybir.dt.float32
    BF16 = mybir.dt.bfloat16
    FP8 = mybir.dt.float8e4
    MULT = AluOpType.mult

    nc = bacc.Bacc("TRN2", target_bir_lowering=False, debug=False)

    # interleaved consumption-order input: [S1c0|E1c0|S1c1|E1c1|...]
    IN = nc.dram_tensor("IN", (NTAG, 2 * NCH * W), FP8, kind="ExternalInput")
    EF = nc.dram_tensor("EF", (NTAG, NTAG), BF16, kind="ExternalInput")
    UOUT = nc.dram_tensor("UOUT", (NTAG, NCH * W), FP8, kind="ExternalOutput")

    with tile.TileContext(nc) as tc, ExitStack() as ctx:
        const = ctx.enter_context(tc.tile_pool(name="const", bufs=1))
        q1p = ctx.enter_context(tc.tile_pool(name="q1p", bufs=2, space="PSUM"))
        q2p = ctx.enter_context(tc.tile_pool(name="q2p", bufs=2, space="PSUM"))
        sp = ctx.enter_context(tc.tile_pool(name="sp", bufs=3))

        ef = const.tile([NTAG, NTAG], BF16, tag="ef", name="ef")
        nc.scalar.dma_start(ef[:], EF[:])

        inbuf = const.tile([NTAG, 2 * NCH * W], FP8, tag="inbuf", name="inbuf")
        ubuf = const.tile([NTAG, NCH * W], FP8, tag="ubuf", name="ubuf")

        # tiered input DMAs in consumption order (units of W columns)
        bounds = [0, 2, 4, 8, 16]
        for k in range(4):
            lo, hi = bounds[k] * W, bounds[k + 1] * W
            nc.sync.dma_start(inbuf[:, lo:hi], IN[:, lo:hi])

        for c in range(NCH):
            s1 = inbuf[:, (2 * c) * W:(2 * c + 1) * W]
            e1 = inbuf[:, (2 * c + 1) * W:(2 * c + 2) * W]
            q1 = q1p.tile([NTAG, W], F32, tag="q1", name=f"q1_{c}")
            nc.tensor.matmul(q1[:, 0:HW], ef[:], s1[:, 0:HW],
                             start=True, stop=True)
            nc.tensor.matmul(q1[:, HW:W], ef[:], s1[:, HW:W],
                             start=True, stop=True)
            s2 = sp.tile([NTAG, W], FP8, tag="s2", name=f"s2_{c}")
            nc.vector.tensor_tensor(s2[:], q1[:], e1, MULT)
            q2 = q2p.tile([NTAG, W], F32, tag="q2", name=f"q2_{c}")
            nc.tensor.matmul(q2[:, 0:HW], ef[:], s2[:, 0:HW],
                             start=True, stop=True)
            nc.tensor.matmul(q2[:, HW:W], ef[:], s2[:, HW:W],
                             start=True, stop=True)
            nc.scalar.copy(ubuf[:, c * W:(c + 1) * W], q2[:])
            if c % 2 == 1:
                lo = (c - 1) * W
                nc.sync.dma_start(UOUT[:, lo:lo + 2 * W],
                                  ubuf[:, lo:lo + 2 * W])

    nc.compile()
    return nc


def _get_program():
    global _PROG
    if _PROG is None:
        _PROG = _build_program()
    return _PROG


def _gold_score(X, y, trans):
    """Gold path score per sequence, float64 on host."""
    Xd = X.astype(np.float64)
    td = trans.astype(np.float64)
    yi = y.astype(np.int64)
    prev = np.concatenate(
        [np.full((B, 1), START, dtype=np.int64), yi[:, :-1]], axis=1
    )
    emit = np.take_along_axis(Xd, yi[:, :, None], axis=2)[:, :, 0]
    tr = td[yi, prev]
    return emit.sum(1) + tr.sum(1) + td[END, yi[:, -1]]


def _prep_in_maps(X, trans):
    bf16 = ml_dtypes.bfloat16
    fp8 = ml_dtypes.float8_e4m3fn
    Tm = np.exp(trans.astype(np.float64) - LNS)       # [i, j]
    efm = np.ascontiguousarray(Tm.T).astype(bf16)     # fwd lhsT
    rho = Tm.sum(axis=1).astype(np.float32)           # T~ @ 1, [128]

    in_maps = []
    for c in range(NCORES):
        Ec = np.exp(X[c * SEQ:(c + 1) * SEQ].astype(np.float32))
        # [tag, blk, t, seq]
        x4 = Ec.transpose(2, 1, 0).reshape(NTAG, NBLK, LB, SEQ)
        s1 = rho[:, None, None] * x4[:, :, 0, :]      # [tag, blk, seq]
        e1h = 0.5 * x4[:, :, 1, :]
        inter = np.empty((NTAG, 2 * NCH, W), dtype=np.float32)
        inter[:, 0::2, :] = s1.reshape(NTAG, NCH, W)
        inter[:, 1::2, :] = e1h.reshape(NTAG, NCH, W)
        xin = np.ascontiguousarray(
            np.clip(inter, 0.0, 240.0).reshape(NTAG, 2 * NCH * W)
        ).astype(fp8)
        in_maps.append({"IN": xin, "EF": efm})
    return in_maps


def kernel(X, y, trans):
    from concourse import bass_utils

    nc = _get_program()
    in_maps = _prep_in_maps(X, trans)
    res = bass_utils.run_bass_kernel_spmd(
        nc, in_maps, core_ids=list(range(NCORES))
    )

    Tm = np.exp(trans.astype(np.float64) - LNS)            # [i, j]
    rho = Tm.sum(axis=1)                                   # [128]
    beta = np.exp(trans[END, :].astype(np.float64) - LNS)  # [128]
    tcol = Tm[:, START]                                    # T~[:, START]

    logZ = np.empty(B, dtype=np.float64)
    for c in range(NCORES):
        # pos b = 0.5 * (T~ @ (e1 .* (T~ @ (rho .* e0)))) of block b
        U = 2.0 * res.results[c]["UOUT"].astype(np.float64).reshape(
            NTAG, NBLK, SEQ)
        Xc = X[c * SEQ:(c + 1) * SEQ].astype(np.float64)   # [32, 1024, 128]
        e0 = np.exp(Xc[:, ::LB, :]).transpose(2, 1, 0)     # [tag, blk, seq]
        e2 = np.exp(Xc[:, 2::LB, :]).transpose(2, 1, 0)
        e3 = np.exp(Xc[:, 3::LB, :]).transpose(2, 1, 0)

        # absorbed block steps 2, 3: u_b = e3 .* (T~ @ (e2 .* U))
        Ufull = e3 * np.einsum("it,tbs->ibs", Tm, e2 * U)
        den = np.einsum("tbs,t->bs", e0, rho)              # [NBLK, SEQ]
        TU = np.einsum("it,tbs->ibs", Tm, Ufull[:, :NBLK - 1, :])
        num = np.empty_like(den)
        num[1:] = np.einsum("tbs,tbs->bs", e0[:, 1:, :], TU)
        num[0] = np.einsum("ts,t->s", e0[:, 0, :], tcol)   # c~_0 . p0
        tail = beta @ Ufull[:, NBLK - 1, :]                # [SEQ]
        lz = (np.log(tail)
              + np.log(num / den).sum(axis=0)
              + (L + 1) * LNS)
        logZ[c * SEQ:(c + 1) * SEQ] = lz

    gold = _gold_score(X, y, trans)
    return (logZ - gold).astype(np.float32)
